# revision 1
# baseline (speedup 1.0000x reference)
"""Trainium2 Bass kernel: single-head self-attention.

Reference computation (fp32):
    q = x @ Wq.T ; k = x @ Wk.T ; v = x @ Wv.T        (x: [4, 2048, 1024])
    out = softmax((q @ k.T) / 32) @ v                 ([4, 2048, 1024])

Sharding: 8 cores = (batch 4) x (sequence halves 2). Each core owns 1024
query rows of one batch element. No collectives: cross-core exchange is
avoided entirely by factoring BOTH sides of the attention through x:
    scores = (x Wq.T)(x Wk.T).T = x (Wq.T Wk) x.T = (x M) x.T
    out    = attn (x Wv.T)      = (attn x) Wv.T
so neither K nor V is ever materialized -- the stationary operands of the
big matmuls are x itself, which every core already holds for the full
sequence. The only replicated work is M = Wq.T Wk (27 us, identical on
every core).

Per-core dataflow (bf16 matmuls with fp32 PSUM accumulation, except the
scores stage which runs fp8-e4m3 DoubleRow):
  - host supplies xT queries-only bf16 ([1024 e, 1024 i]), the full
    reordered xT in fp8-e4m3 ([1024 e, 2048 s], loaded into pair-plane
    [128, 2, S] tiles for DoubleRow), the reordered x row-major bf16
    ([2048 s, 1024 e]), the ORIGINAL Wq/Wk ([f, e]), and Wv.T ([e, f]).
  - M[e,e'] = sum_f Wq[f,e] Wk[f,e'] : 128 bf16 matmuls (f on partitions).
  - yT[e',i] = sum_e M[e,e'] xt[e,i] : 128 bf16 matmuls over own queries,
    drained straight to fp8 pair-plane tiles.
  - scoresT[j,i] = sum_e' xt8[e',j] yt8[e',i] : 32 chains of 4 DoubleRow
    fp8 matmuls (two 128-row contraction planes per instruction, charged
    0.5 cycles/row -- 4x the bf16 chain rate). Score-side fp8 error is
    attenuated through softmax (score error -> relative attention error):
    measured 1.80e-2 vs the 2e-2 gate. V-side operands must stay bf16
    (their error hits the output linearly at ~3.6%/operand).
  - ScalarE applies exp(scores/32) straight out of PSUM (max-subtraction
    unnecessary: |scores/32| < ~2.5 by construction); 5 zT accumulation
    groups are held in PSUM and accumulated one j-tile behind the scores
    chains so the PE fills the exp-drain windows instead of stalling.
  - denominators per query arrive in [i-partition, 1] layout via
    expT.T @ ones matmuls accumulated across j-tiles in one PSUM tile.
  - zT[e,i] = sum_j xr[j,e] expT[j,i] : 256 bf16 matmuls total
    (attn @ x over the full key sequence).
  - out[i,f] = (sum_e zT[e,i] wv[e,f]) * recip[i] : 128 bf16 matmuls;
    normalization folds into the drain as a per-partition scale on the
    PSUM->SBUF copy (it commutes through the linear Wv projection).

Performance: ~153 us of gap-free TensorE streaming at 2.4 GHz (640 bf16
N=512 matmuls + 128 fp8 DoubleRow matmuls + 128 N=1 denominator matmuls);
softmax/drains run on ScalarE/VectorE under the matmul stream; DMA loads
overlap the M phase; PE p-state warmup burns the clock ramp inside the
startup DMA window.
"""

import numpy as np
import ml_dtypes
from contextlib import ExitStack

import concourse.bacc as bacc
import concourse.tile as tile
import concourse.mybir as mybir

BF16 = mybir.dt.bfloat16
FP8 = mybir.dt.float8e4
F32 = mybir.dt.float32
P = 128
B, S, D = 4, 2048, 1024
SQ = S // 2  # query rows per core
N_CORES = 8
ET = D // P   # contraction tiles over embed dim
FT = D // P   # feature tiles
JT = S // P   # kv-sequence tiles
IT = SQ // P  # query tiles
NCH = 512     # moving-operand chunk (one fp32 PSUM bank)
INV_SQRT_D = 1.0 / 32.0

_CACHE: dict = {}


def _build(repeats=1):
    nc = bacc.Bacc("TRN2", target_bir_lowering=False, debug=False, num_devices=N_CORES)
    xt = nc.dram_tensor("xt", [D, SQ], BF16, kind="ExternalInput").ap()
    xt8 = nc.dram_tensor("xt8", [D, S], FP8, kind="ExternalInput").ap()
    xr = nc.dram_tensor("xr", [S, D], BF16, kind="ExternalInput").ap()
    wqo = nc.dram_tensor("wqo", [D, D], BF16, kind="ExternalInput").ap()
    wko = nc.dram_tensor("wko", [D, D], BF16, kind="ExternalInput").ap()
    wv = nc.dram_tensor("wv", [D, D], BF16, kind="ExternalInput").ap()
    out = nc.dram_tensor("out", [SQ, D], F32, kind="ExternalOutput").ap()

    with tile.TileContext(nc) as tc, ExitStack() as ctx:
        xt_pool = ctx.enter_context(tc.tile_pool(name="xt", bufs=1))
        xt8_pool = ctx.enter_context(tc.tile_pool(name="xt8", bufs=1))
        xr_pool = ctx.enter_context(tc.tile_pool(name="xr", bufs=1))
        w_pool = ctx.enter_context(tc.tile_pool(name="w", bufs=1))
        m_pool = ctx.enter_context(tc.tile_pool(name="m", bufs=1))
        yt_pool = ctx.enter_context(tc.tile_pool(name="yt", bufs=1))
        zt_pool = ctx.enter_context(tc.tile_pool(name="zt", bufs=1))
        exp_pool = ctx.enter_context(tc.tile_pool(name="expT", bufs=1))
        stage_pool = ctx.enter_context(tc.tile_pool(name="stage", bufs=4))
        small_pool = ctx.enter_context(tc.tile_pool(name="small", bufs=1))
        mm_psum = ctx.enter_context(tc.tile_pool(name="mmps", bufs=7, space="PSUM"))
        dn_psum = ctx.enter_context(tc.tile_pool(name="dnps", bufs=1, space="PSUM"))

        # load order matches compute order: M needs wqo+wko (interleaved
        # per-ft pairs so the ft-outer M phase streams at DMA arrival rate),
        # then xt (yT at ~32 us), then xr (zT at ~137 us) and wv (out-proj)
        def load_w(wap, tagname):
            tiles = []
            for et in range(ET):
                t = w_pool.tile([P, D], BF16, name=f"{tagname}{et}")
                nc.sync.dma_start(t[:], wap[et * P:(et + 1) * P, :])
                tiles.append(t)
            return tiles

        wqo_sb, wko_sb = [], []
        for et in range(ET):
            tq = w_pool.tile([P, D], BF16, name=f"wqo{et}")
            if et == 0:
                nc.sync.dma_start(tq[:, 0:896], wqo[0:P, 0:896])
            else:
                nc.sync.dma_start(tq[:], wqo[et * P:(et + 1) * P, :])
            wqo_sb.append(tq)
            tk = w_pool.tile([P, D], BF16, name=f"wko{et}")
            if et == 0:
                # SWDGE path: Pool-engine descgen runs concurrently with the
                # HWDGE descgen of the wqo0 slice, so both first-chain
                # operands land ~1 us earlier
                nc.gpsimd.dma_start(tk[:, 0:NCH], wko[0:P, 0:NCH])
            else:
                nc.sync.dma_start(tk[:], wko[et * P:(et + 1) * P, :])
            wko_sb.append(tk)
        # remainders of the first pair (needed only by M part B / later)
        nc.sync.dma_start(wqo_sb[0][:, 896:D], wqo[0:P, 896:D])
        nc.sync.dma_start(wko_sb[0][:, NCH:D], wko[0:P, NCH:D])
        xt_sb = []
        for et in range(ET):
            t = xt_pool.tile([P, SQ], BF16, name=f"xt{et}")
            nc.sync.dma_start(t[:], xt[et * P:(et + 1) * P, :])
            xt_sb.append(t)
        xt8_sb = []
        for g in range(ET // 2):
            t = xt8_pool.tile([P, 2, S], FP8, name=f"xt8_{g}")
            nc.sync.dma_start(t[:, 0, :], xt8[(2 * g) * P:(2 * g + 1) * P, :])
            nc.sync.dma_start(t[:, 1, :], xt8[(2 * g + 1) * P:(2 * g + 2) * P, :])
            xt8_sb.append(t)
        xr_sb = []
        for jt in range(JT):
            t = xr_pool.tile([P, D], BF16, name=f"xr{jt}")
            nc.sync.dma_start(t[:], xr[jt * P:(jt + 1) * P, :])
            xr_sb.append(t)
        wv_sb = load_w(wv, "wv")

      # (indentation block below runs once per repeat; repeats>1 is a
      # timing-only configuration)
        for _rep in range(repeats):
            _compute(nc, tc, ctx, xt_sb, xt8_sb, xr_sb, wqo_sb, wko_sb, wv_sb,
                     m_pool, yt_pool, zt_pool, exp_pool, stage_pool, small_pool,
                     mm_psum, dn_psum, out)

    nc.compile()
    return nc


def _compute(nc, tc, ctx, xt_sb, xt8_sb, xr_sb, wqo_sb, wko_sb, wv_sb,
             m_pool, yt_pool, zt_pool, exp_pool, stage_pool, small_pool,
             mm_psum, dn_psum, out):
    # ---- Phase W: PE p-state warmup. The cost model (and HW) run the PE at
    # half clock until ~3 us of continuous busy; burn that ramp on dummy
    # const matmuls during the otherwise-idle startup DMA window so the real
    # stream starts at full speed.
    warm_c = nc.const_aps.tensor(1.0, (P, NCH), BF16)
    for w in range(7):
        psw = mm_psum.tile([P, NCH], F32, name="ps_w", tag="mm")
        nc.tensor.matmul(psw[:], warm_c[:, 0:P], warm_c[:], start=True, stop=True)

    # ---- Phase M: M[e, e'] = sum_f Wq[f, e] Wk[f, e']
    # Part A runs ft-outermost with 7 concurrent PSUM accumulation groups so
    # the PE streams as soon as the first (wqo, wko) tile pair lands, at the
    # DMA arrival cadence, instead of stalling until all weights are loaded.
    m_sb = [m_pool.tile([P, D], BF16, name=f"m{mt}") for mt in range(ET)]
    ps_a = [mm_psum.tile([P, NCH], F32, name=f"ps_mA{mt}", tag="mm")
            for mt in range(7)]
    for ft in range(ET):
        for mt in range(7):
            nc.tensor.matmul(
                ps_a[mt][:],
                wqo_sb[ft][:, mt * P:(mt + 1) * P],
                wko_sb[ft][:, 0:NCH],
                start=(ft == 0),
                stop=(ft == ET - 1),
            )
    for mt in range(7):
        nc.vector.tensor_copy(m_sb[mt][:, 0:NCH], ps_a[mt][:])
    for mt, nch in [(7, 0)] + [(mt, 1) for mt in range(ET)]:
        ps = mm_psum.tile([P, NCH], F32, name="ps_m", tag="mm")
        for ft in range(ET):
            nc.tensor.matmul(
                ps[:],
                wqo_sb[ft][:, mt * P:(mt + 1) * P],
                wko_sb[ft][:, nch * NCH:(nch + 1) * NCH],
                start=(ft == 0),
                stop=(ft == ET - 1),
            )
        nc.vector.tensor_copy(m_sb[mt][:, nch * NCH:(nch + 1) * NCH], ps[:])

    # ---- Phase Y: yT[e', i] = sum_e M[e, e'] xt[e, i]  (queries = cols [0, SQ))
    # drained straight to fp8e4m3 in pair-plane layout for the DoubleRow
    # scores matmuls
    yt8_sb = [yt_pool.tile([P, 2, SQ], FP8, name=f"yt8_{g}") for g in range(FT // 2)]
    for ft in range(FT):
        for ic in range(SQ // NCH):
            ps = mm_psum.tile([P, NCH], F32, name="ps_y", tag="mm")
            for et in range(ET):
                nc.tensor.matmul(
                    ps[:],
                    m_sb[et][:, ft * P:(ft + 1) * P],
                    xt_sb[et][:, ic * NCH:(ic + 1) * NCH],
                    start=(et == 0),
                    stop=(et == ET - 1),
                )
            nc.vector.tensor_copy(
                yt8_sb[ft // 2][:, ft % 2, ic * NCH:(ic + 1) * NCH], ps[:])

    # ---- Phase B: expT[j, i] = exp(scoresT/32), scoresT[j,i] = sum_e' xt[e',j] yT[e',i]
    # 5 zT accumulation groups (et 0..4, ic=0) are held in PSUM across the
    # scores loop and accumulated one jt behind the scores chains -- the PE
    # does zT work during the exp ACT drains instead of stalling on the PSUM
    # rotation (the fp8 scores chains outrun the Activation engine).
    exp_sb = [exp_pool.tile([P, SQ], BF16, name=f"expT{jt}") for jt in range(JT)]
    NZH = 5
    zt_sb = [zt_pool.tile([P, SQ], BF16, name=f"zt{et}") for et in range(ET)]
    ps_z = [mm_psum.tile([P, NCH], F32, name=f"ps_zh{g}", tag="mm")
            for g in range(NZH)]

    def z_held_mms(jt):
        for g in range(NZH):
            nc.tensor.matmul(
                ps_z[g][:],
                xr_sb[jt][:, g * P:(g + 1) * P],
                exp_sb[jt][:, 0:NCH],
                start=(jt == 0),
                stop=(jt == JT - 1),
            )

    for jt in range(JT):
        for ic in range(SQ // NCH):
            ps = mm_psum.tile([P, NCH], F32, name="ps_s", tag="mm")
            for g in range(FT // 2):
                nc.tensor.matmul(
                    ps[:],
                    xt8_sb[g][:, :, jt * P:(jt + 1) * P],
                    yt8_sb[g][:, :, ic * NCH:(ic + 1) * NCH],
                    start=(g == 0),
                    stop=(g == FT // 2 - 1),
                    perf_mode=mybir.MatmulPerfMode.DoubleRow,
                )
            nc.scalar.activation(
                exp_sb[jt][:, ic * NCH:(ic + 1) * NCH],
                ps[:],
                mybir.ActivationFunctionType.Exp,
                scale=INV_SQRT_D,
            )
        if jt >= 1:
            z_held_mms(jt - 1)
    z_held_mms(JT - 1)
    for g in range(NZH):
        nc.vector.tensor_copy(zt_sb[g][:, 0:NCH], ps_z[g][:])

    # ---- Phase B2: denomT[i(part), it] via expT.T @ ones, one PSUM tile with
    # a separate accumulation group per column; recipT = 1/denomT
    ones_bf16 = nc.const_aps.tensor(1.0, (P, 1), BF16)
    dn = dn_psum.tile([P, IT], F32, name="ps_dn")
    for it in range(IT):
        for jt in range(JT):
            nc.tensor.matmul(
                dn[:, it:it + 1],
                exp_sb[jt][:, it * P:(it + 1) * P],
                ones_bf16,
                start=(jt == 0),
                stop=(jt == JT - 1),
            )
    denomT = small_pool.tile([P, IT], F32, name="denomT")
    recipT = small_pool.tile([P, IT], F32, name="recipT")
    nc.vector.tensor_copy(denomT[:], dn[:])
    nc.vector.reciprocal(recipT[:], denomT[:])

    # ---- Phase Z: remaining zT groups (the first NZH of ic=0 were computed
    # interleaved with the scores loop above)
    for et in range(ET):
        for ic in range(SQ // NCH):
            if ic == 0 and et < NZH:
                continue
            ps = mm_psum.tile([P, NCH], F32, name="ps_z", tag="mm")
            for jt in range(JT):
                nc.tensor.matmul(
                    ps[:],
                    xr_sb[jt][:, et * P:(et + 1) * P],
                    exp_sb[jt][:, ic * NCH:(ic + 1) * NCH],
                    start=(jt == 0),
                    stop=(jt == JT - 1),
                )
            nc.vector.tensor_copy(zt_sb[et][:, ic * NCH:(ic + 1) * NCH], ps[:])

    # ---- Phase C: out[i, f] = (sum_e zT[e, i] wv[e, f]) * recip[i]
    # The very last chunk is split (288+224, sim-swept) so the final
    # ACT-drain + descgen + DMA + sem tail after the last matmul is short.
    chunks = [(it, fc * NCH, NCH) for it in range(IT) for fc in range(D // NCH)]
    chunks = chunks[:-1] + [(IT - 1, D - NCH, 288), (IT - 1, D - 224, 224)]
    for it, f0, fw in chunks:
        ps = mm_psum.tile([P, fw], F32, name="ps_o", tag="mm")
        for et in range(ET):
            nc.tensor.matmul(
                ps[:],
                zt_sb[et][:, it * P:(it + 1) * P],
                wv_sb[et][:, f0:f0 + fw],
                start=(et == 0),
                stop=(et == ET - 1),
            )
        st = stage_pool.tile([P, fw], F32, name="ostage", tag="ostage")
        nc.scalar.activation(
            st[:],
            ps[:],
            mybir.ActivationFunctionType.Copy,
            scale=recipT[:, it:it + 1],
        )
        nc.sync.dma_start(out[it * P:(it + 1) * P, f0:f0 + fw], st[:])


def _get_nc(repeats=1):
    key = ("nc", repeats)
    if key not in _CACHE:
        _CACHE[key] = _build(repeats)
    return _CACHE[key]


def _prep_inputs(x, Wq, Wk, Wv):
    bf16 = ml_dtypes.bfloat16
    x = np.asarray(x, dtype=np.float32)
    wq_o = np.ascontiguousarray(np.asarray(Wq, dtype=np.float32).astype(bf16))
    wk_o = np.ascontiguousarray(np.asarray(Wk, dtype=np.float32).astype(bf16))
    wv_t = np.ascontiguousarray(np.asarray(Wv, dtype=np.float32).T.astype(bf16))
    in_maps = []
    for c in range(N_CORES):
        b, h = divmod(c, 2)
        xb = x[b].astype(bf16)  # [S, D]
        # this core's query half first, then the other half (j-order is a
        # consistent permutation of the keys and values, so attention is
        # unaffected)
        xr = np.concatenate([xb[h * SQ:(h + 1) * SQ], xb[(1 - h) * SQ:(2 - h) * SQ]], axis=0)
        xr_c = np.ascontiguousarray(xr)    # [S, D]
        xt_full = xr.T                     # [D, S]
        xt_c = np.ascontiguousarray(xt_full[:, 0:SQ])  # bf16 queries only (yT moving)
        xt8_c = np.ascontiguousarray(
            xt_full.astype(np.float32).astype(ml_dtypes.float8_e4m3))  # fp8 full seq
        in_maps.append({"xt": xt_c, "xt8": xt8_c, "xr": xr_c,
                        "wqo": wq_o, "wko": wk_o, "wv": wv_t})
    return in_maps


def _get_runner():
    """Cached jitted dispatcher: one XLA/NEFF compile per process, reused
    across kernel() calls (run_bass_kernel_spmd would recompile per call)."""
    if "runner" in _CACHE:
        return _CACHE["runner"]
    import jax
    from jax.sharding import Mesh, PartitionSpec
    from jax.experimental.shard_map import shard_map
    from concourse.bass2jax import (
        _bass_exec_p, install_neuronx_cc_hook, partition_id_tensor)

    nc = _get_nc()
    install_neuronx_cc_hook()

    in_names, out_names, out_avals = [], [], []
    partition_name = nc.partition_id_tensor.name if nc.partition_id_tensor else None
    for alloc in nc.m.functions[0].allocations:
        if not isinstance(alloc, mybir.MemoryLocationSet):
            continue
        name = alloc.memorylocations[0].name
        if alloc.kind == "ExternalInput":
            if name != partition_name:
                in_names.append(name)
        elif alloc.kind == "ExternalOutput":
            out_names.append(name)
            out_avals.append(jax.core.ShapedArray(
                tuple(alloc.tensor_shape), mybir.dt.np(alloc.dtype)))
    n_params = len(in_names)
    all_names = list(in_names) + out_names
    if partition_name is not None:
        all_names.append(partition_name)

    def _body(*args):
        operands = list(args)
        if partition_name is not None:
            operands.append(partition_id_tensor())
        return tuple(_bass_exec_p.bind(
            *operands,
            out_avals=tuple(out_avals),
            in_names=tuple(all_names),
            out_names=tuple(out_names),
            lowering_input_output_aliases=(),
            sim_require_finite=True,
            sim_require_nnan=True,
            nc=nc,
        ))

    devices = jax.devices()[:N_CORES]
    mesh = Mesh(np.asarray(devices), ("core",))
    nspecs = (PartitionSpec("core"),) * (n_params + len(out_names))
    sharded = jax.jit(
        shard_map(_body, mesh=mesh, in_specs=nspecs,
                  out_specs=(PartitionSpec("core"),) * len(out_names),
                  check_rep=False),
        keep_unused=True,
    )

    def run(in_maps):
        concat_in = [
            np.concatenate([in_maps[c][name] for c in range(N_CORES)], axis=0)
            for name in in_names
        ]
        concat_zero = [
            np.zeros((N_CORES * a.shape[0], *a.shape[1:]), a.dtype)
            for a in out_avals
        ]
        outs = sharded(*concat_in, *concat_zero)
        return {
            name: np.asarray(outs[i]).reshape(N_CORES, *out_avals[i].shape)
            for i, name in enumerate(out_names)
        }

    _CACHE["runner"] = run
    return run


def kernel(x, Wq, Wk, Wv):
    in_maps = _prep_inputs(x, Wq, Wk, Wv)
    res = _get_runner()(in_maps)
    out = np.empty((B, S, D), dtype=np.float32)
    for c in range(N_CORES):
        b, h = divmod(c, 2)
        out[b, h * SQ:(h + 1) * SQ, :] = res["out"][c]
    return out



# revision 50
# speedup vs baseline: 1.2170x; 1.2170x over previous
"""Trainium2 Bass kernel: single-head self-attention.

Reference computation (fp32):
    q = x @ Wq.T ; k = x @ Wk.T ; v = x @ Wv.T        (x: [4, 2048, 1024])
    out = softmax((q @ k.T) / 32) @ v                 ([4, 2048, 1024])

Sharding: 8 cores = (batch 4) x (sequence halves 2). Each core owns 1024
query rows of one batch element. No collectives: cross-core exchange is
avoided entirely by factoring BOTH sides of the attention through x:
    scores = (x Wq.T)(x Wk.T).T = x (Wq.T Wk) x.T = (x M) x.T
    out    = attn (x Wv.T)      = (attn x) Wv.T
so neither K nor V is ever materialized -- the stationary operands of the
big matmuls are x itself, which every core already holds for the full
sequence.

Every matmul runs fp8-e4m3 DoubleRow (0.5 cycles per moving row, 4x the
bf16 chain rate, 256-deep contraction planes). bf16-level accuracy where
needed comes from hi/lo operand splitting: X ~= X_hi + X_lo with
X_hi = fp8(X), X_lo = fp8(X - X_hi), so X (.) A = X_hi A_hi + X_hi A_lo
+ X_lo A_hi (the lo*lo term is ~2^-8 relative and dropped). A 3-pass
split matmul costs 0.75x the bf16 cycles at matching accuracy; all three
passes accumulate into one PSUM chain so drains are unchanged. The
attention-score side tolerates single-fp8 operands only where measured:
scores stay 1-pass (xt8 (.) yt8, error attenuated through softmax); M
and Y run 3-pass (2-pass variants measured over the 2e-2 gate). The V
side (Z = attn@x, C = z@Wv.T) is linear in operand error and runs
3-pass. Measured end-to-end rel-absmax 1.70e-2 vs the 2e-2 gate (the
bf16 baseline measured 1.80e-2).

All scale factors are powers of two (error-free): weights are quantized
at 32x (their +-1/32 range would be subnormal in e4m3), xr at 1/4 (so
zT = z/4 stays under the 240 e4m3 max), y drains at 2^-5, the exp
activation folds the combined 2^-10 into its scale argument, and the
denominator matmul uses ones=8.0 so the final per-query reciprocal
absorbs the z/4 * 32Wv = 8x output scale.

Per-core dataflow (all matmuls fp8 DoubleRow with fp32 PSUM):
  - hi/lo splits of all inputs are prepared host-side and shipped as
    merged multi-plane DRAM tensors so each SBUF tile loads with ONE
    wide DMA (24 input DMAs total -- HWDGE issue costs ~650 ns SEQ +
    625 ns descgen each, so many small plane loads would starve the PE).
  - M'[e,e'] = (32Wq).T (32Wk) 3-pass: batch 1 runs all 8 nch=0 tiles
    as concurrent PSUM groups (7 from the mm pool + 1 borrowed from the
    denominator bank) f-group-outermost at DMA arrival rate; batch 2
    (nch=1) chains after. Drained to m8 hi/lo by VectorE.
  - yT'[e',i] = M'8 (.) xt8 3-pass over own queries, ACT-drained at 2^-5
    straight to fp8 yt8 (single).
  - scoresT[j,i] = xt8hi (.) yt8, 1-pass; ScalarE applies exp(s * 2^-10)
    out of PSUM to a bf16 stage; VectorE extracts e8hi = fp8(E) and
    e8lo = fp8(E - e8hi). 5 zT accumulation groups are held in PSUM and
    their 3-pass matmuls are spread a few-per-chain behind the scores
    stream so the PE fills every exp-drain window.
  - zT[e,i] = xr8 (.) e8 3-pass over the full key sequence; the 64
    denominator DoubleRow N=1 matmuls (e8hi (.) 8.0 into one PSUM tile)
    are sprinkled between Z chains where the PE SEQ has slack, so they
    cost nothing.
  - out[i,f] = (z8 (.) wv8 3-pass) * recip[i]; normalization folds into
    the ACT drain as a per-partition scale. The last chunk is split
    (288+224) and its stores issue from the Activation queue right
    after their drains to shorten the final DMA tail.

Performance: ~283k gap-free TensorE cycles at 2.4 GHz; DMA (13 MB, fp8
everywhere) overlaps the M phase; softmax/drains run on ScalarE/VectorE
under the matmul stream; PE p-state warmup burns the clock ramp inside
the startup DMA window.
"""

import numpy as np
import ml_dtypes
from contextlib import ExitStack

import concourse.bacc as bacc
import concourse.tile as tile
import concourse.mybir as mybir

BF16 = mybir.dt.bfloat16
FP8 = mybir.dt.float8e4
F32 = mybir.dt.float32
P = 128
B, S, D = 4, 2048, 1024
SQ = S // 2   # query rows per core
N_CORES = 8
ET = D // P   # contraction tiles over embed dim
GE = ET // 2  # pair-plane groups over embed dim
JT = S // P   # kv-sequence tiles
GJ = JT // 2  # pair-plane groups over kv sequence
IT = SQ // P  # query tiles
NCH = 512     # moving-operand chunk (one fp32 PSUM bank)
DR = mybir.MatmulPerfMode.DoubleRow
NZH = 5       # zT accumulation groups held in PSUM under the scores loop

_CACHE: dict = {}


def _build(repeats=1, upto=99):
    nc = bacc.Bacc("TRN2", target_bir_lowering=False, debug=False, num_devices=N_CORES)
    # merged multi-plane inputs: one wide DMA per SBUF tile
    wqk8 = nc.dram_tensor("wqk8", [GE, P, 8, D], FP8, kind="ExternalInput").ap()
    xtq8 = nc.dram_tensor("xtq8", [GE, P, 4, SQ], FP8, kind="ExternalInput").ap()
    xtk8 = nc.dram_tensor("xtk8", [GE, P, 2, SQ], FP8, kind="ExternalInput").ap()
    xr8 = nc.dram_tensor("xr8", [GJ, P, 4, D], FP8, kind="ExternalInput").ap()
    wv8 = nc.dram_tensor("wv8", [GE, P, 4, D], FP8, kind="ExternalInput").ap()
    out = nc.dram_tensor("out", [SQ, D], F32, kind="ExternalOutput").ap()

    with tile.TileContext(nc) as tc, ExitStack() as ctx:
        w_pool = ctx.enter_context(tc.tile_pool(name="w", bufs=1))
        xt_pool = ctx.enter_context(tc.tile_pool(name="xt", bufs=1))
        xr_pool = ctx.enter_context(tc.tile_pool(name="xr", bufs=1))
        wv_pool = ctx.enter_context(tc.tile_pool(name="wv", bufs=1))
        m_pool = ctx.enter_context(tc.tile_pool(name="m", bufs=1))
        yt_pool = ctx.enter_context(tc.tile_pool(name="yt", bufs=1))
        e_pool = ctx.enter_context(tc.tile_pool(name="e", bufs=1))
        z_pool = ctx.enter_context(tc.tile_pool(name="z", bufs=1))
        estage_pool = ctx.enter_context(tc.tile_pool(name="estage", bufs=6))
        stage_pool = ctx.enter_context(tc.tile_pool(name="stage", bufs=6))
        small_pool = ctx.enter_context(tc.tile_pool(name="small", bufs=1))
        mm_psum = ctx.enter_context(tc.tile_pool(name="mmps", bufs=7, space="PSUM"))
        dn_psum = ctx.enter_context(tc.tile_pool(name="dnps", bufs=1, space="PSUM"))

        # ---- DMA loads: one dma_start per tile, issued on the SP queue in
        # first-use order (the serial DMA bus then transfers in this order).
        # hi planes land first so each group's pass-0 matmuls start ~1.75 us
        # earlier than a single 1 MB quad transfer would allow
        wqk_sb = []
        for g in range(GE):
            t = w_pool.tile([P, 8, D], FP8, name=f"wqk{g}")
            nc.sync.dma_start(t[:, 0:4, :], wqk8[g][:, 0:4, :])
            nc.sync.dma_start(t[:, 4:8, :], wqk8[g][:, 4:8, :])
            wqk_sb.append(t)
        xtq_sb, xtk_sb, xr_sb, wv_sb = [], [], [], []
        if upto >= 2:
            for g in range(GE):
                t = xt_pool.tile([P, 4, SQ], FP8, name=f"xtq{g}")
                nc.sync.dma_start(t[:], xtq8[g])
                xtq_sb.append(t)
        if upto >= 3:
            for g in range(GE):
                t = xt_pool.tile([P, 2, SQ], FP8, name=f"xtk{g}")
                nc.sync.dma_start(t[:], xtk8[g])
                xtk_sb.append(t)
            for gj in range(GJ):
                t = xr_pool.tile([P, 4, D], FP8, name=f"xr{gj}")
                nc.sync.dma_start(t[:], xr8[gj])
                xr_sb.append(t)
        if upto >= 5:
            for g in range(GE):
                t = wv_pool.tile([P, 4, D], FP8, name=f"wv{g}")
                nc.sync.dma_start(t[:], wv8[g])
                wv_sb.append(t)

        tensors = dict(wqk=wqk_sb, xtq=xtq_sb, xtk=xtk_sb, xr=xr_sb, wv=wv_sb)
        for _rep in range(repeats):
            _compute(nc, tensors, m_pool, yt_pool, e_pool, z_pool,
                     estage_pool, stage_pool, small_pool, mm_psum, dn_psum, out,
                     upto=upto)

    nc.compile()
    return nc


def _compute(nc, t, m_pool, yt_pool, e_pool, z_pool, estage_pool, stage_pool,
             small_pool, mm_psum, dn_psum, out, upto=99):
    Exp = mybir.ActivationFunctionType.Exp
    Copy = mybir.ActivationFunctionType.Copy
    sub = mybir.AluOpType.subtract
    # plane slices of the merged weight tiles
    QHI, KHI, QLO, KLO = slice(0, 2), slice(2, 4), slice(4, 6), slice(6, 8)
    HI, LO = slice(0, 2), slice(2, 4)

    # ---- Phase W: PE p-state warmup. The cost model (and HW) run the PE at
    # reduced clock until ~3 us of continuous busy; burn that ramp on dummy
    # const matmuls during the otherwise-idle startup DMA window.
    warm_c = small_pool.tile([P, NCH], BF16, name="warm_c")
    nc.vector.memset(warm_c[:], 1.0)
    for w in range(8):
        psw = mm_psum.tile([P, NCH], F32, name="ps_w", tag="mm")
        nc.tensor.matmul(psw[:], warm_c[:, 0:P], warm_c[:], start=True, stop=True)
    ones8 = small_pool.tile([P, 2, 1], FP8, name="ones8")
    nc.vector.memset(ones8[:], 8.0)

    # ---- Phase M: M'[e, e'] = sum_f (32Wq)[f, e] (32Wk)[f, e'], 3-pass.
    # Batch 1 (nch=0) runs all 8 e-tiles as concurrent PSUM groups
    # f-group-outermost so the PE streams at the DMA arrival cadence; the
    # 8th group borrows the denominator bank (idle until phase B2).
    mhi_sb = [m_pool.tile([P, 2, D], FP8, name=f"mhi{g}") for g in range(GE)]
    mlo_sb = [m_pool.tile([P, 2, D], FP8, name=f"mlo{g}") for g in range(GE)]
    m_passes = [(QHI, KHI), (QHI, KLO), (QLO, KHI)]

    # hi extraction rides the otherwise-idle Activation engine; only the lo
    # subtract stays on VectorE (a PSUM-reading DVE op costs ~658 ns per
    # [P,512] chunk -- both on one engine would pace the whole M phase)
    def m_drain(mt, nch, ps):
        hi = mhi_sb[mt // 2][:, mt % 2, nch * NCH:(nch + 1) * NCH]
        lo = mlo_sb[mt // 2][:, mt % 2, nch * NCH:(nch + 1) * NCH]
        nc.scalar.activation(hi, ps[:], Copy, scale=1.0)
        nc.vector.tensor_tensor(lo, ps[:], hi, sub)

    # the denominator bank moonlights as the 8th M accumulation group (it
    # is idle until phase B2, and the start flag resets accumulation)
    ps_dn_bank = dn_psum.tile([P, NCH], F32, name="ps_dn")
    ps_a = [mm_psum.tile([P, NCH], F32, name=f"ps_mA{mt}", tag="mm")
            for mt in range(7)]
    ps_a.append(ps_dn_bank)
    def m_mm(ps, g, sq, sk, mt, nch, start, stop):
        nc.tensor.matmul(
            ps[:],
            t["wqk"][g][:, sq, mt * P:(mt + 1) * P],
            t["wqk"][g][:, sk, nch * NCH:(nch + 1) * NCH],
            start=start, stop=stop, perf_mode=DR,
        )

    for g in range(GE - 1):
        for pi, (sq, sk) in enumerate(m_passes):
            for mt in range(ET):
                m_mm(ps_a[mt], g, sq, sk, mt, 0, g == 0 and pi == 0, False)
    # last f-group runs tile-major with immediate drains so PSUM banks free
    # one by one (batch 2 starts ~2 us earlier than an all-tiles-at-g3 end)
    for mt in range(ET):
        for pi, (sq, sk) in enumerate(m_passes):
            m_mm(ps_a[mt], GE - 1, sq, sk, mt, 0, False, pi == len(m_passes) - 1)
        m_drain(mt, 0, ps_a[mt])
    for mt in range(ET):
        ps = mm_psum.tile([P, NCH], F32, name="ps_m", tag="mm")
        for pi, (sq, sk) in enumerate(m_passes):
            for g in range(GE):
                m_mm(ps, g, sq, sk, mt, 1,
                     g == 0 and pi == 0,
                     g == GE - 1 and pi == len(m_passes) - 1)
        m_drain(mt, 1, ps)

    if upto < 2:
        return
    # ---- Phase Y: yT'[e', i] = sum_e M'[e, e'] xt[e, i], 3-pass, queries
    # only; ACT-drained at 2^-5 straight to fp8 yt8 for the scores matmuls.
    yt8_sb = [yt_pool.tile([P, 2, SQ], FP8, name=f"yt8_{g}") for g in range(GE)]
    y_passes = [(mhi_sb, HI), (mlo_sb, HI), (mhi_sb, LO)]
    for ft in range(ET):
        for ic in range(SQ // NCH):
            ps = mm_psum.tile([P, NCH], F32, name="ps_y", tag="mm")
            for pi, (msb, xsl) in enumerate(y_passes):
                for g in range(GE):
                    nc.tensor.matmul(
                        ps[:],
                        msb[g][:, :, ft * P:(ft + 1) * P],
                        t["xtq"][g][:, xsl, ic * NCH:(ic + 1) * NCH],
                        start=(g == 0 and pi == 0),
                        stop=(g == GE - 1 and pi == len(y_passes) - 1),
                        perf_mode=DR,
                    )
            nc.scalar.activation(
                yt8_sb[ft // 2][:, ft % 2, ic * NCH:(ic + 1) * NCH],
                ps[:], Copy, scale=2.0 ** -5)

    if upto < 3:
        return
    # ---- Phase S: scoresT[j, i] = sum_e' xt8[e', j] yt8[e', i] (1-pass);
    # exp via ScalarE to a bf16 stage, VectorE extracts e8 hi/lo. NZH zT
    # accumulation groups are held in PSUM; their 3-pass matmuls are
    # emitted a few per scores chain (eligibility lags the exp drains by
    # one chain) so the PE fills every drain window.
    ehi_sb = [e_pool.tile([P, 2, SQ], FP8, name=f"ehi{gj}") for gj in range(GJ)]
    elo_sb = [e_pool.tile([P, 2, SQ], FP8, name=f"elo{gj}") for gj in range(GJ)]
    zhi_sb = [z_pool.tile([P, 2, SQ], FP8, name=f"zhi{g}") for g in range(GE)]
    zlo_sb = [z_pool.tile([P, 2, SQ], FP8, name=f"zlo{g}") for g in range(GE)]
    # allocated lazily at the first held-z matmul so the early filler-less
    # scores chains can rotate through all 7 mm banks
    ps_z = []
    z_passes = [(HI, ehi_sb), (LO, ehi_sb), (HI, elo_sb)]

    def z_mm(i, ic, gj, pi):
        if not ps_z:
            ps_z.extend(mm_psum.tile([P, NCH], F32, name=f"ps_zh{k}", tag="mm")
                        for k in range(NZH))
        xsl, esb = z_passes[pi]
        nc.tensor.matmul(
            ps_z[i][:],
            t["xr"][gj][:, xsl, i * P:(i + 1) * P],
            esb[gj][:, :, ic * NCH:(ic + 1) * NCH],
            start=(gj == 0 and pi == 0),
            stop=(gj == GJ - 1 and pi == len(z_passes) - 1),
            perf_mode=DR,
        )

    # Pending PE filler matmuls, emitted a few per scores chain: the held-z
    # accumulations plus the denominator chains (denomT[i(part), it] via
    # e8hi (.) 8.0, DoubleRow N=1 into the dn bank, one accumulation group
    # per column -- SEQ-bound, so they ride the stream's decode slack).
    # PSUM supports only one open accumulation chain per bank, so each
    # (it, gj) partial is its own single-shot chain into its own column
    # (col = it*GJ + gj); a VectorE tree-add reduces [P, IT*GJ] -> [P, IT].
    dn = ps_dn_bank

    def dn_mm(it, gj):
        col = it * GJ + gj
        nc.tensor.matmul(
            dn[:, col:col + 1],
            ehi_sb[gj][:, :, it * P:(it + 1) * P],
            ones8[:],
            start=True, stop=True,
            perf_mode=DR,
        )

    held = []
    for gj in range(GJ):
        if upto >= 4:
            held.extend(("z", gj, pi, i) for pi in range(3) for i in range(NZH))
        if upto >= 5:
            held.extend(("dn", gj, it, None) for it in range(IT))
    emitted = 0

    def emit_held(jt, ic, budget):
        nonlocal emitted
        # group gj's exp tiles are complete after chain (2gj+1, ic=1); allow
        # release half a chain early (ic=1 of the same jt) so the queue never
        # runs dry at odd-jt frontier boundaries
        while emitted < len(held) and budget > 0:
            kind, gj, a, b = held[emitted]
            if 2 * gj + 1 >= jt:
                return
            if kind == "z":
                z_mm(b, 0, gj, a)
            else:
                dn_mm(a, gj)
            emitted += 1
            budget -= 1

    for jt in range(JT):
        for ic in range(SQ // NCH):
            ps = mm_psum.tile([P, NCH], F32, name="ps_s", tag="mm")
            for g in range(GE):
                stat = (t["xtq"][g][:, HI, jt * P:(jt + 1) * P] if jt < IT
                        else t["xtk"][g][:, :, (jt - IT) * P:(jt - IT + 1) * P])
                nc.tensor.matmul(
                    ps[:], stat,
                    yt8_sb[g][:, :, ic * NCH:(ic + 1) * NCH],
                    start=(g == 0), stop=(g == GE - 1),
                    perf_mode=DR,
                )
            est = estage_pool.tile([P, NCH], BF16, name="estage", tag="est")
            nc.scalar.activation(est[:], ps[:], Exp, scale=2.0 ** -10)
            ehi = ehi_sb[jt // 2][:, jt % 2, ic * NCH:(ic + 1) * NCH]
            elo = elo_sb[jt // 2][:, jt % 2, ic * NCH:(ic + 1) * NCH]
            nc.vector.tensor_copy(ehi, est[:])
            nc.vector.tensor_tensor(elo, est[:], ehi, sub)
            emit_held(jt, ic, 5)
    while emitted < len(held):
        kind, gj, a, b = held[emitted]
        if kind == "z":
            z_mm(b, 0, gj, a)
        else:
            dn_mm(a, gj)
        emitted += 1
    if upto < 5:
        return
    add = mybir.AluOpType.add
    dnv = dn[:, 0:IT * GJ].rearrange("p (a b) -> p a b", a=IT, b=GJ)
    ds8 = small_pool.tile([P, IT, GJ], F32, name="ds8")
    ds4 = small_pool.tile([P, IT, 4], F32, name="ds4")
    ds2 = small_pool.tile([P, IT, 2], F32, name="ds2")
    denomT = small_pool.tile([P, IT], F32, name="denomT")
    recipT = small_pool.tile([P, IT], F32, name="recipT")
    nc.vector.tensor_copy(ds8[:], dnv)
    nc.vector.tensor_tensor(ds4[:], ds8[:, :, 0:4], ds8[:, :, 4:8], add)
    nc.vector.tensor_tensor(ds2[:], ds4[:, :, 0:2], ds4[:, :, 2:4], add)
    nc.vector.tensor_tensor(denomT[:], ds2[:, :, 0], ds2[:, :, 1], add)
    nc.vector.reciprocal(recipT[:], denomT[:])

    def z_drain(et, ic, ps):
        hi = zhi_sb[et // 2][:, et % 2, ic * NCH:(ic + 1) * NCH]
        lo = zlo_sb[et // 2][:, et % 2, ic * NCH:(ic + 1) * NCH]
        nc.scalar.activation(hi, ps[:], Copy, scale=1.0)
        nc.vector.tensor_tensor(lo, ps[:], hi, sub)

    for i in range(NZH):
        z_drain(i, 0, ps_z[i])

    # ---- Phase Z: remaining zT groups (the first NZH of ic=0 were computed
    # interleaved with the scores loop above)
    for et in range(ET):
        for ic in range(SQ // NCH):
            if ic == 0 and et < NZH:
                continue
            ps = mm_psum.tile([P, NCH], F32, name="ps_z", tag="mm")
            for pi in range(3):
                for gj in range(GJ):
                    xsl, esb = z_passes[pi]
                    nc.tensor.matmul(
                        ps[:],
                        t["xr"][gj][:, xsl, et * P:(et + 1) * P],
                        esb[gj][:, :, ic * NCH:(ic + 1) * NCH],
                        start=(gj == 0 and pi == 0),
                        stop=(gj == GJ - 1 and pi == 2),
                        perf_mode=DR,
                    )
            z_drain(et, ic, ps)

    # ---- Phase C: out[i, f] = (sum_e zT[e, i] wv[e, f]) * recip[i], 3-pass.
    # The very last chunk is split so the final ACT-drain + descgen + DMA +
    # sem tail after the last matmul is short; the last two stores issue
    # from the Activation queue straight after their drains.
    # chunk order is free (all recips ready): chunks depending on the LAST
    # z-drain (it 4..7, ic1 region) run mid-stream so its ACT+sem latency
    # hides under earlier chains; the stream opens with early-drained it 0..2
    # and closes on it3 (ic0, drained long before) with a tiny 64-col chunk
    # so the final drain+store tail is short
    c_passes = [(zhi_sb, HI), (zhi_sb, LO), (zlo_sb, HI)]
    early = [0, 1, 2, 4, 5, 6, 7]
    chunks = [(it, fc * NCH, NCH) for it in early for fc in range(D // NCH)]
    chunks += [(3, 0, NCH), (3, NCH, 448), (3, D - 64, 64)]
    for ci, (it, f0, fw) in enumerate(chunks):
        ps = mm_psum.tile([P, fw], F32, name="ps_o", tag="mm")
        for pi, (zsb, wsl) in enumerate(c_passes):
            for g in range(GE):
                nc.tensor.matmul(
                    ps[:],
                    zsb[g][:, :, it * P:(it + 1) * P],
                    t["wv"][g][:, wsl, f0:f0 + fw],
                    start=(g == 0 and pi == 0),
                    stop=(g == GE - 1 and pi == len(c_passes) - 1),
                    perf_mode=DR,
                )
        st = stage_pool.tile([P, fw], F32, name="ostage", tag="ostage")
        nc.scalar.activation(st[:], ps[:], Copy, scale=recipT[:, it:it + 1])
        nc.sync.dma_start(out[it * P:(it + 1) * P, f0:f0 + fw], st[:])


def _get_nc(repeats=1):
    key = ("nc", repeats)
    if key not in _CACHE:
        _CACHE[key] = _build(repeats)
    return _CACHE[key]


def _prep_inputs(x, Wq, Wk, Wv):
    fp8 = ml_dtypes.float8_e4m3
    f32 = np.float32

    def split8(a):
        hi = a.astype(fp8)
        lo = (a - hi.astype(f32)).astype(fp8)
        return hi, lo

    def pair_planes(*mats):
        """Stack [D_rows, C] fp8 mats into [groups, P, nplanes, C]: for each
        row-pair-group g, planes are (m0 rows 2g, m0 rows 2g+1, m1 rows 2g,
        m1 rows 2g+1, ...)."""
        rows = mats[0].shape[0]
        ng = rows // (2 * P)
        cols = mats[0].shape[1]
        outp = np.empty((ng, P, 2 * len(mats), cols), fp8)
        for g in range(ng):
            for mi, m in enumerate(mats):
                outp[:, :, 2 * mi + 0, :][g] = m[(2 * g) * P:(2 * g + 1) * P]
                outp[:, :, 2 * mi + 1, :][g] = m[(2 * g + 1) * P:(2 * g + 2) * P]
        return outp

    x = np.asarray(x, dtype=f32)
    wq_hi, wq_lo = split8(32.0 * np.asarray(Wq, f32))
    wk_hi, wk_lo = split8(32.0 * np.asarray(Wk, f32))
    wv_hi, wv_lo = split8(np.ascontiguousarray(32.0 * np.asarray(Wv, f32).T))
    wqk = pair_planes(wq_hi, wk_hi, wq_lo, wk_lo)   # [GE, P, 8, D]
    wv8 = pair_planes(wv_hi, wv_lo)                 # [GE, P, 4, D]
    in_maps = []
    for c in range(N_CORES):
        b, h = divmod(c, 2)
        xb = x[b]  # [S, D]
        # this core's query half first, then the other half (j-order is a
        # consistent permutation of the keys and values, so attention is
        # unaffected)
        xr = np.concatenate([xb[h * SQ:(h + 1) * SQ], xb[(1 - h) * SQ:(2 - h) * SQ]],
                            axis=0)          # [S, D]
        xt = np.ascontiguousarray(xr.T)      # [D, S]
        xt_hi, xt_lo = split8(xt)
        xr_hi, xr_lo = split8(np.ascontiguousarray(xr) * 0.25)
        in_maps.append({
            "wqk8": wqk,
            "xtq8": pair_planes(xt_hi[:, 0:SQ], xt_lo[:, 0:SQ]),  # [GE, P, 4, SQ]
            "xtk8": pair_planes(xt_hi[:, SQ:S]),                  # [GE, P, 2, SQ]
            "xr8": pair_planes(xr_hi, xr_lo),                     # [GJ, P, 4, D]
            "wv8": wv8,
        })
    return in_maps


def _get_runner():
    """Cached jitted dispatcher: one XLA/NEFF compile per process, reused
    across kernel() calls (run_bass_kernel_spmd would recompile per call)."""
    if "runner" in _CACHE:
        return _CACHE["runner"]
    import jax
    from jax.sharding import Mesh, PartitionSpec
    from jax.experimental.shard_map import shard_map
    from concourse.bass2jax import (
        _bass_exec_p, install_neuronx_cc_hook, partition_id_tensor)

    nc = _get_nc()
    install_neuronx_cc_hook()

    in_names, out_names, out_avals = [], [], []
    partition_name = nc.partition_id_tensor.name if nc.partition_id_tensor else None
    for alloc in nc.m.functions[0].allocations:
        if not isinstance(alloc, mybir.MemoryLocationSet):
            continue
        name = alloc.memorylocations[0].name
        if alloc.kind == "ExternalInput":
            if name != partition_name:
                in_names.append(name)
        elif alloc.kind == "ExternalOutput":
            out_names.append(name)
            out_avals.append(jax.core.ShapedArray(
                tuple(alloc.tensor_shape), mybir.dt.np(alloc.dtype)))
    n_params = len(in_names)
    all_names = list(in_names) + out_names
    if partition_name is not None:
        all_names.append(partition_name)

    def _body(*args):
        operands = list(args)
        if partition_name is not None:
            operands.append(partition_id_tensor())
        return tuple(_bass_exec_p.bind(
            *operands,
            out_avals=tuple(out_avals),
            in_names=tuple(all_names),
            out_names=tuple(out_names),
            lowering_input_output_aliases=(),
            sim_require_finite=True,
            sim_require_nnan=True,
            nc=nc,
        ))

    devices = jax.devices()[:N_CORES]
    mesh = Mesh(np.asarray(devices), ("core",))
    nspecs = (PartitionSpec("core"),) * (n_params + len(out_names))
    sharded = jax.jit(
        shard_map(_body, mesh=mesh, in_specs=nspecs,
                  out_specs=(PartitionSpec("core"),) * len(out_names),
                  check_rep=False),
        keep_unused=True,
    )

    def run(in_maps):
        concat_in = [
            np.concatenate([in_maps[c][name] for c in range(N_CORES)], axis=0)
            for name in in_names
        ]
        concat_zero = [
            np.zeros((N_CORES * a.shape[0], *a.shape[1:]), a.dtype)
            for a in out_avals
        ]
        outs = sharded(*concat_in, *concat_zero)
        return {
            name: np.asarray(outs[i]).reshape(N_CORES, *out_avals[i].shape)
            for i, name in enumerate(out_names)
        }

    _CACHE["runner"] = run
    return run


def kernel(x, Wq, Wk, Wv):
    in_maps = _prep_inputs(x, Wq, Wk, Wv)
    res = _get_runner()(in_maps)
    out = np.empty((B, S, D), dtype=np.float32)
    for c in range(N_CORES):
        b, h = divmod(c, 2)
        out[b, h * SQ:(h + 1) * SQ, :] = res["out"][c]
    return out
